# revision 21
# baseline (speedup 1.0000x reference)
"""Trainium2 Bass kernel for nn_DeepFM_3066606649824.

Strategy (8 NeuronCores, data-parallel over batch):
  - Host: restructure the 26 FFM embedding tables [26, 208000, 16] f32 into one
    bf16 row-major table G2 [208000, 432]: col 0 = fm1_emb, cols 8:424 = the 26
    tables' rows concatenated (table-major). One gathered row then serves the
    fm1 sum, the FFM second-order products, and the DNN input.
  - Each core takes 512 batch rows and gathers its 512*26 = 13312 rows with
    indirect DMA (864B/row), issued FIELD-major so downstream compute pipelines
    under the gather window (the gather is SWDGE-issue-bound on the Pool
    engine, ~1.4us per 128-row gather => ~146us; everything else must hide
    under it).
  - fm2 via DVE scalar_tensor_tensor with an i<j access pattern on the raw
    gathered rows (half the work of the full-matrix + diagonal version);
    op (s, j) only needs fields 0..j so it runs as soon as field j lands.
  - DNN: PE-transposes g chunks to [feature, batch], the dense-path
    rd.T = relu(dWr_chunk.T @ Xd.T) is computed per chunk on PE (K=14 matmul)
    and folded during the PSUM->SBUF staging STT (relu(rd)+gT), then h1.T
    accumulates over 87 chunks on PE in bf16 with f32 PSUM. The PE stream is
    software-pipelined (transpose chunk ci, then matmul chunk ci-1).
  - BatchNorm stats are all-reduced across the 8 cores (exact); a dummy
    warmup AllReduce at t0 absorbs first-collective latency.
"""

import os
import sys

for _p in ("/opt/trn_rl_repo",):
    if _p not in sys.path and os.path.isdir(_p):
        sys.path.insert(0, _p)

import numpy as np
import ml_dtypes

from concourse import bass, mybir
import concourse.tile as tile
from concourse.vector_clock import ScopedClock
from concourse.bass_utils import run_bass_kernel_spmd
from concourse.masks import make_identity

BF16 = mybir.dt.bfloat16
F32 = mybir.dt.float32
I32 = mybir.dt.int32
AF = mybir.ActivationFunctionType
OP = mybir.AluOpType

# N_CORES only controls how many cores run (replica groups / in_maps);
# the per-core shard is fixed at BS/8. N_CORES<8 is a debug mode where only
# the first N_CORES shards are computed (BN stats then cover only those).
N_CORES = int(os.environ.get("DFM_N_CORES", "8"))
F = 26
V_FIELD = 8000
V = F * V_FIELD            # 208000
D = 16
FD = F * D                 # 416
ROW = 432                  # padded G2 row: [fm1, 7 pad, 416 feats, 8 pad]
FEAT_OFF = 8
DNN_IN = F * F * D         # 10816
H1, H2 = 256, 128
BS = 4096
SHARD = BS // 8            # 512
NS = SHARD // 128          # batch sub-tiles of 128
NDENSE = 13
EPS = 1e-5

# K-chunk map for the main matmul, ordered so every chunk is ready as soon as
# its field(s) are gathered: per tail-group t (fields 3t..3t+2): the fields'
# three full 128-row chunks each, then the packed 32-row tails chunk.
TAIL_GROUP = 3
N_TAIL = (F + TAIL_GROUP - 1) // TAIL_GROUP      # 9
CHUNKS = []  # (kind, payload): ("full", (j, piece)) | ("tail", t)
for _t in range(N_TAIL):
    for _u in range(min(TAIL_GROUP, F - TAIL_GROUP * _t)):
        for _p in range(3):
            CHUNKS.append(("full", (TAIL_GROUP * _t + _u, _p)))
    CHUNKS.append(("tail", _t))
N_CHUNKS = len(CHUNKS)     # 87


def _chunk_k(kind, payload):
    if kind == "full":
        return 128
    t = payload
    return 32 * min(TAIL_GROUP, F - TAIL_GROUP * t)


def _chunk_rows(ci):
    """Feature indices (in W1p g-order) for chunk ci's rows."""
    kind, payload = CHUNKS[ci]
    if kind == "full":
        j, p = payload
        return list(range(j * FD + 128 * p, j * FD + 128 * (p + 1)))
    t = payload
    rows = []
    for u in range(min(TAIL_GROUP, F - TAIL_GROUP * t)):
        j = TAIL_GROUP * t + u
        rows.extend(range(j * FD + 384, j * FD + FD))
    return rows


def _install_drain_split():
    """This container's walrus rejects >1 sync-wait per TPB_CTRL instruction;
    split the Tile kernel-tail drain's waits onto single-wait NOPs."""
    if getattr(tile.TileContext, "_dfm_drain_patched", False):
        return

    def _split_drain_and_barrier(self, tick_clock, wait_clock):
        collector = self.nc.sync.nop(nofuse=True)
        wait_clock.add_sem_waits(
            collector.ins, ScopedClock({None: tick_clock.global_clock})
        )
        si = collector.ins.sync_info
        waits = list(si.on_wait) if si is not None else []
        if len(waits) > 1:
            si.on_wait = waits[:1]
            for i in range(1, len(waits)):
                extra = self.nc.sync.nop(nofuse=True)
                extra.ins.sync_info = mybir.SyncInfo(
                    on_wait=[waits[i]], on_update=[]
                )
        self.nc.sync.drain()
        self.nc.all_engine_barrier()
        assert self.sems is not None
        popped = self.nc._tile_sem_poison_stack.pop()
        assert popped is self._sem_poison
        self.nc.clear_and_free_semaphores(list(self.sems.allocated().values()))
        self.nc.all_engine_barrier()

    tile.TileContext._drain_and_barrier = _split_drain_and_barrier
    tile.TileContext._dfm_drain_patched = True


def _split_multiwaits(nc, max_waits=1):
    """This walrus build also rejects >1 sync-wait on regular engine
    instructions: hoist extra waits onto single-wait NOPs just before."""
    n_split = 0
    for fn in nc.m.functions:
        for bb in fn.blocks:
            new_insts = []
            for inst in bb.instructions:
                si = getattr(inst, "sync_info", None)
                waits = list(si.on_wait) if si is not None and si.on_wait else []
                if len(waits) > max_waits:
                    keep = waits[-max_waits:]
                    for k, w in enumerate(waits[:-max_waits]):
                        nop = mybir.InstNoOp(
                            name=f"{inst.name}_w{k}",
                            engine=inst.engine,
                            sync_info=mybir.SyncInfo(
                                on_wait=[w], on_update=[]
                            ),
                            bass_nofuse=True,
                        )
                        new_insts.append(nop)
                    si.on_wait = keep
                    n_split += 1
                new_insts.append(inst)
            bb.instructions[:] = new_insts
    return n_split


def build_program(split_waits=True):
    _install_drain_split()
    nc = bass.Bass()

    g2_d = nc.declare_dram_parameter("g2", [V, ROW], BF16, isOutput=False)
    idx_d = nc.declare_dram_parameter("idx", [128, NS * F], I32, isOutput=False)
    w1_d = nc.declare_dram_parameter("w1", [128, N_CHUNKS * H1], BF16, isOutput=False)
    dwrk_d = nc.declare_dram_parameter(
        "dwrk", [NDENSE + 1, N_CHUNKS * 128], BF16, isOutput=False)
    xdt_d = nc.declare_dram_parameter("xdt", [NDENSE + 1, SHARD], BF16, isOutput=False)
    w2_d = nc.declare_dram_parameter("w2", [128, H1], BF16, isOutput=False)
    wout_d = nc.declare_dram_parameter("wout", [128, 1], BF16, isOutput=False)
    fm1w_d = nc.declare_dram_parameter("fm1w", [NDENSE, 1], BF16, isOutput=False)
    bn1g_d = nc.declare_dram_parameter("bn1g", [128, 2], F32, isOutput=False)
    bn1b_d = nc.declare_dram_parameter("bn1b", [128, 2], F32, isOutput=False)
    bn2g_d = nc.declare_dram_parameter("bn2g", [128, 1], F32, isOutput=False)
    bn2b_d = nc.declare_dram_parameter("bn2b", [128, 1], F32, isOutput=False)
    c0_d = nc.declare_dram_parameter("c0", [128, 1], F32, isOutput=False)
    out_d = nc.declare_dram_parameter("out", [SHARD, 1], F32, isOutput=True)
    debug_taps = bool(int(os.environ.get("DFM_DEBUG", "0")))
    if debug_taps:
        dbg_d = nc.declare_dram_parameter(
            "dbg", [128, NS * F + 40], F32, isOutput=True)
        dbg_h_d = nc.declare_dram_parameter(
            "dbg_h", [128, 2 * SHARD], F32, isOutput=True)
        dbg2_d = nc.declare_dram_parameter(
            "dbg2", [128, 12], F32, isOutput=True)

    groups = [list(range(N_CORES))]

    with tile.TileContext(nc) as tc:
        with (
            tc.tile_pool(name="persist", bufs=1) as persist,
            tc.tile_pool(name="gbuf", bufs=1) as gbuf,
            tc.tile_pool(name="scr", bufs=1) as scrp,
            tc.tile_pool(name="stage", bufs=8) as stagep,
            tc.tile_pool(name="rdst", bufs=8) as rdstp,
            tc.tile_pool(name="small", bufs=2) as small,
            tc.tile_pool(name="ps_h1", bufs=1, space="PSUM") as ps_h1,
            tc.tile_pool(name="ps_stage", bufs=2, space="PSUM") as ps_stage,
            tc.tile_pool(name="ps_rd", bufs=2, space="PSUM") as ps_rd,
            tc.tile_pool(name="ps_small", bufs=1, space="PSUM") as ps_small,
            tc.tile_pool(name="dram", bufs=1, space="DRAM") as dram,
        ):
            # ---- warmup AllReduce (absorbs first-collective latency) ----
            wu_in = dram.tile([128, 1], F32, tag="wui")
            wu_out = dram.tile([128, 1], F32, tag="wuo")
            nc.gpsimd.collective_compute(
                "AllReduce", OP.add, replica_groups=groups,
                ins=[wu_in.opt()], outs=[wu_out.opt()],
            )

            # ---- load constants / weights ----
            ident = persist.tile([128, 128], BF16)
            make_identity(nc, ident[:])

            idx = persist.tile([128, NS * F], I32, tag="idx")
            nc.sync.dma_start(idx[:], idx_d[:])
            xdt = persist.tile([NDENSE + 1, SHARD], BF16, tag="xdt")
            nc.sync.dma_start(xdt[:], xdt_d[:])
            dwrk = persist.tile([NDENSE + 1, N_CHUNKS * 128], BF16, tag="dwrk")
            nc.sync.dma_start(dwrk[:], dwrk_d[:])
            w1 = persist.tile([128, N_CHUNKS * H1], BF16, tag="w1")
            # w1 is 5.7MB; load in 4 slices so the first chunks' weights
            # arrive quickly and spread across queues.
            for q in range(4):
                sl = slice(q * (N_CHUNKS * H1 // 4), (q + 1) * (N_CHUNKS * H1 // 4))
                nc.sync.dma_start(w1[:, sl], w1_d[:, sl])
            w2 = persist.tile([128, H1], BF16, tag="w2")
            nc.sync.dma_start(w2[:], w2_d[:])
            wout = persist.tile([128, 1], BF16, tag="wout")
            nc.sync.dma_start(wout[:], wout_d[:])
            fm1w = persist.tile([NDENSE, 1], BF16, tag="fm1w")
            nc.sync.dma_start(fm1w[:], fm1w_d[:])
            bn1g = persist.tile([128, 2], F32, tag="bn1g")
            nc.sync.dma_start(bn1g[:], bn1g_d[:])
            bn1b = persist.tile([128, 2], F32, tag="bn1b")
            nc.sync.dma_start(bn1b[:], bn1b_d[:])
            bn2g = persist.tile([128, 1], F32, tag="bn2g")
            nc.sync.dma_start(bn2g[:], bn2g_d[:])
            bn2b = persist.tile([128, 1], F32, tag="bn2b")
            nc.sync.dma_start(bn2b[:], bn2b_d[:])
            c0 = persist.tile([128, 1], F32, tag="c0")
            nc.sync.dma_start(c0[:], c0_d[:])

            g = [gbuf.tile([128, F * ROW], BF16, tag=f"g{s}", name=f"g{s}")
                 for s in range(NS)]

            # head: the dense fm1 part is independent of the gather; run early.
            head_ps = ps_small.tile([128, 2 * NS], F32, tag="heads")
            for s in range(NS):
                nc.tensor.matmul(
                    head_ps[:, NS + s:NS + s + 1],
                    lhsT=xdt[0:NDENSE, s * 128:(s + 1) * 128],
                    rhs=fm1w[:],
                    start=True, stop=True,
                )

            # ---- pipelined gather + fm2 + transpose/fold/matmul ----
            h1_ps = [
                ps_h1.tile([128, SHARD], F32, tag=f"h1_{h}", name=f"h1_ps{h}")
                for h in range(2)
            ]
            Spart = small.tile([128, NS * F], F32, tag="fm2Spart")
            scr = scrp.tile([128, FD], BF16, tag="fm2scr")
            part0 = None

            pend = []          # chunks transposed+folded but not yet matmuled
            rdq = {}           # ci -> (rdr tile, kc), rd pipeline lookahead
            RD_LOOKAHEAD = 6
            PEND_DEPTH = 4

            def emit_rd(ci):
                """rd (dense-path) pipeline: PE K=14 matmul + Act relu to
                SBUF bf16. Independent of the gather, so it runs ahead."""
                kind, payload = CHUNKS[ci]
                kc = _chunk_k(kind, payload)
                rd_ps = ps_rd.tile([128, SHARD], F32, tag="rd")
                nc.tensor.matmul(
                    rd_ps[0:kc, :],
                    lhsT=dwrk[:, ci * 128: ci * 128 + kc],
                    rhs=xdt[:, :],
                    start=True, stop=True,
                )
                rdr = rdstp.tile([128, SHARD], BF16, tag="rdr")
                nc.scalar.activation(
                    out=rdr[0:kc, :], in_=rd_ps[0:kc, :], func=AF.Relu,
                )
                rdq[ci] = (rdr, kc)

            def emit_mm(ci, rhs, kc):
                nc.tensor.matmul(
                    h1_ps[0][:],
                    lhsT=w1[0:kc, ci * H1 + 0 * 128: ci * H1 + 1 * 128],
                    rhs=rhs[0:kc, :],
                    start=(ci == 0), stop=(ci == N_CHUNKS - 1),
                )
                nc.tensor.matmul(
                    h1_ps[1][:],
                    lhsT=w1[0:kc, ci * H1 + 1 * 128: ci * H1 + 2 * 128],
                    rhs=rhs[0:kc, :],
                    start=(ci == 0), stop=(ci == N_CHUNKS - 1),
                )
                nc.tensor.ldweights(ident[:])  # HAM keep-warm

            def emit_chunk(ci):
                """Transpose chunk ci into PSUM, fold with the (pre-computed)
                rd into SBUF rhs; defer its h1 matmuls by PEND_DEPTH chunks
                (PE pipeline skew)."""
                if ci + RD_LOOKAHEAD < N_CHUNKS:
                    emit_rd(ci + RD_LOOKAHEAD)
                kind, payload = CHUNKS[ci]
                kc = _chunk_k(kind, payload)
                st_ps = ps_stage.tile([128, SHARD], BF16, tag="st")
                if kind == "full":
                    j, p = payload
                    off = j * ROW + FEAT_OFF + 128 * p
                    for s in range(NS):
                        nc.tensor.transpose(
                            out=st_ps[0:128, s * 128:(s + 1) * 128],
                            in_=g[s][:, off:off + 128],
                            identity=ident[:],
                        )
                else:
                    t = payload
                    for u in range(min(TAIL_GROUP, F - TAIL_GROUP * t)):
                        j = TAIL_GROUP * t + u
                        off = j * ROW + FEAT_OFF + 384
                        for s in range(NS):
                            nc.tensor.transpose(
                                out=st_ps[32 * u:32 * (u + 1),
                                          s * 128:(s + 1) * 128],
                                in_=g[s][:, off:off + 32],
                                identity=ident[:],
                            )
                # rhs = gT + relu(rd); rdr was staged ahead by the rd pipeline
                rdr, kc2 = rdq.pop(ci)
                assert kc2 == kc
                nc.tensor.ldweights(ident[:])  # HAM keep-warm, no output
                rhs = stagep.tile([128, SHARD], BF16, tag="rhs")
                nc.vector.tensor_tensor(
                    out=rhs[0:kc, :], in0=st_ps[0:kc, :], in1=rdr[0:kc, :],
                    op=OP.add,
                )
                pend.append((ci, rhs, kc))
                if len(pend) > PEND_DEPTH:
                    emit_mm(*pend.pop(0))

            for ci0 in range(min(RD_LOOKAHEAD, N_CHUNKS)):
                emit_rd(ci0)

            ci_iter = 0
            for t in range(N_TAIL):
                nf = min(TAIL_GROUP, F - TAIL_GROUP * t)
                # all the group's gathers first (writers must be emitted
                # before any reader for Tile dep tracking)
                for u in range(nf):
                    j = TAIL_GROUP * t + u
                    for s in range(NS):
                        nc.gpsimd.indirect_dma_start(
                            out=g[s][:, j * ROW:(j + 1) * ROW],
                            out_offset=None,
                            in_=g2_d[:, :],
                            in_offset=bass.IndirectOffsetOnAxis(
                                ap=idx[:, s * F + j:s * F + j + 1], axis=0
                            ),
                        )
                if part0 is None:
                    part0 = list(g[0][:].ap[0])
                # fm2 cross terms for field j against fields i<j:
                #   Spart[:, j] = sum_{i<j,d} g[p, j, 16i+d] * g[p, i, 16j+d]
                for u in range(nf):
                    j = TAIL_GROUP * t + u
                    if j == 0:
                        continue
                    for s in range(NS):
                        g_s = g[s]
                        in0 = bass.AP(
                            g_s[:].tensor,
                            g_s[:].offset + j * ROW + FEAT_OFF,
                            [part0, [D, j], [1, D]],
                        )
                        in1 = bass.AP(
                            g_s[:].tensor,
                            g_s[:].offset + FEAT_OFF + D * j,
                            [part0, [ROW, j], [1, D]],
                        )
                        nc.vector.scalar_tensor_tensor(
                            out=scr[:, 0:j * D].rearrange(
                                "p (i d) -> p i d", d=D),
                            in0=in0, scalar=1.0, in1=in1,
                            op0=OP.mult, op1=OP.mult,
                            accum_out=Spart[:, s * F + j:s * F + j + 1],
                        )
                for _ in range(3 * nf + 1):
                    emit_chunk(ci_iter)
                    ci_iter += 1
            assert ci_iter == N_CHUNKS
            while pend:
                emit_mm(*pend.pop(0))

            # ---- fm1 (embedding part) and fm2 reduction ----
            fm1e = small.tile([128, NS], F32, tag="fm1e")
            S_acc = small.tile([128, NS], F32, tag="fm2S")
            for s in range(NS):
                fm1_ap = g[s][:].rearrange("p (j e) -> p j e", j=F)[:, :, 0]
                nc.vector.tensor_reduce(
                    out=fm1e[:, s:s + 1], in_=fm1_ap,
                    axis=mybir.AxisListType.X, op=OP.add,
                )
                nc.vector.tensor_reduce(
                    out=S_acc[:, s:s + 1],
                    in_=Spart[:, s * F + 1:(s + 1) * F],
                    axis=mybir.AxisListType.X, op=OP.add,
                )

            # ---- BN1 stats + allreduce ----
            stats1 = small.tile([128, 4], F32, tag="stats1")
            sq_scr = scrp.tile([128, SHARD], F32, tag="sq")
            for h in range(2):
                nc.vector.tensor_reduce(
                    out=stats1[:, h:h + 1], in_=h1_ps[h][:],
                    axis=mybir.AxisListType.X, op=OP.add,
                )
                nc.scalar.activation(
                    out=sq_scr[:], in_=h1_ps[h][:], func=AF.Square,
                    accum_out=stats1[:, 2 + h:3 + h],
                )
            b1_in = dram.tile([128, 4], F32, tag="b1i")
            b1_out = dram.tile([128, 4], F32, tag="b1o")
            nc.sync.dma_start(b1_in[:], stats1[:])
            nc.gpsimd.collective_compute(
                "AllReduce", OP.add, replica_groups=groups,
                ins=[b1_in.opt()], outs=[b1_out.opt()],
            )
            stats1g = small.tile([128, 4], F32, tag="stats1g")
            nc.sync.dma_start(stats1g[:], b1_out[:])

            def bn_scale_bias(statsg, col_s, col_q, gamma, beta, ncols):
                # returns (scale, bias) [128, ncols] f32
                mean = small.tile([128, ncols], F32, tag="bn_mean")
                var = small.tile([128, ncols], F32, tag="bn_var")
                scale = small.tile([128, ncols], F32, tag="bn_scale")
                bias = small.tile([128, ncols], F32, tag="bn_bias")
                tmp = small.tile([128, ncols], F32, tag="bn_tmp")
                nc.vector.tensor_scalar_mul(
                    mean[:], statsg[:, col_s:col_s + ncols], 1.0 / BS)
                nc.vector.tensor_scalar_mul(
                    var[:], statsg[:, col_q:col_q + ncols], 1.0 / BS)
                nc.vector.tensor_tensor(
                    out=tmp[:], in0=mean[:], in1=mean[:], op=OP.mult)
                nc.vector.tensor_tensor(
                    out=var[:], in0=var[:], in1=tmp[:], op=OP.subtract)
                nc.vector.tensor_scalar_add(var[:], var[:], EPS)
                nc.vector.reciprocal(tmp[:], var[:])
                nc.scalar.activation(out=tmp[:], in_=tmp[:], func=AF.Sqrt)
                nc.vector.tensor_tensor(
                    out=scale[:], in0=gamma[:], in1=tmp[:], op=OP.mult)
                nc.vector.tensor_tensor(
                    out=tmp[:], in0=mean[:], in1=scale[:], op=OP.mult)
                nc.vector.tensor_tensor(
                    out=bias[:], in0=beta[:], in1=tmp[:], op=OP.subtract)
                return scale, bias

            sc1, bi1 = bn_scale_bias(stats1g, 0, 2, bn1g, bn1b, 2)
            h1r = persist.tile([128, 2, SHARD], BF16, tag="h1r")
            for h in range(2):
                nc.scalar.activation(
                    out=h1r[:, h, :], in_=h1_ps[h][:], func=AF.Relu,
                    bias=bi1[:, h:h + 1], scale=sc1[:, h:h + 1],
                )

            # ---- layer 2 ----
            h2_ps = ps_small.tile([128, SHARD], F32, tag="h2")
            for h in range(2):
                nc.tensor.matmul(
                    h2_ps[:],
                    lhsT=w2[:, h * 128:(h + 1) * 128],
                    rhs=h1r[:, h, :],
                    start=(h == 0), stop=(h == 1),
                )
            stats2 = small.tile([128, 2], F32, tag="stats2")
            nc.vector.tensor_reduce(
                out=stats2[:, 0:1], in_=h2_ps[:],
                axis=mybir.AxisListType.X, op=OP.add,
            )
            sq_scr2 = scrp.tile([128, SHARD], F32, tag="sq")
            nc.scalar.activation(
                out=sq_scr2[:], in_=h2_ps[:], func=AF.Square,
                accum_out=stats2[:, 1:2],
            )
            b2_in = dram.tile([128, 2], F32, tag="b2i")
            b2_out = dram.tile([128, 2], F32, tag="b2o")
            nc.sync.dma_start(b2_in[:], stats2[:])
            nc.gpsimd.collective_compute(
                "AllReduce", OP.add, replica_groups=groups,
                ins=[b2_in.opt()], outs=[b2_out.opt()],
            )
            stats2g = small.tile([128, 2], F32, tag="stats2g")
            nc.sync.dma_start(stats2g[:], b2_out[:])
            sc2, bi2 = bn_scale_bias(stats2g, 0, 1, bn2g, bn2b, 1)
            h2r = persist.tile([128, SHARD], BF16, tag="h2r")
            nc.scalar.activation(
                out=h2r[:], in_=h2_ps[:], func=AF.Relu,
                bias=bi2[:, 0:1], scale=sc2[:, 0:1],
            )

            # ---- heads ----
            for s in range(NS):
                nc.tensor.matmul(
                    head_ps[:, s:s + 1],
                    lhsT=h2r[:, s * 128:(s + 1) * 128],
                    rhs=wout[:],
                    start=True, stop=True,
                )

            tot = small.tile([128, NS], F32, tag="tot")
            res = small.tile([128, NS], F32, tag="res")
            nc.vector.tensor_tensor(
                out=tot[:], in0=fm1e[:], in1=head_ps[:, 0:NS], op=OP.add)
            nc.vector.tensor_tensor(
                out=tot[:], in0=tot[:], in1=head_ps[:, NS:2 * NS], op=OP.add)
            nc.vector.tensor_tensor(
                out=tot[:], in0=tot[:], in1=S_acc[:], op=OP.add)
            nc.scalar.activation(
                out=res[:], in_=tot[:], func=AF.Sigmoid,
                bias=c0[:, 0:1], scale=1.0,
            )
            out_ap = out_d[:, :].rearrange("(s p) o -> p (s o)", p=128)
            nc.sync.dma_start(out_ap, res[:])

            if debug_taps:
                dbg = scrp.tile([128, NS * F + 40], F32, tag="dbg")
                nc.vector.tensor_copy(dbg[:, 0:NS * F], Spart[:])
                nc.vector.tensor_copy(
                    dbg[:, NS * F:NS * F + NS], fm1e[:])
                nc.vector.tensor_copy(
                    dbg[:, NS * F + NS:NS * F + 2 * NS], S_acc[:])
                nc.vector.tensor_copy(
                    dbg[:, NS * F + 8:NS * F + 12], stats1[:])
                nc.vector.tensor_copy(
                    dbg[:, NS * F + 12:NS * F + 16], stats1g[:])
                nc.vector.tensor_copy(
                    dbg[:, NS * F + 16:NS * F + 16 + 2 * NS],
                    head_ps[:, 0:2 * NS])
                o = NS * F + 24
                nc.vector.tensor_copy(dbg[:, o:o + 2], sc1[:])
                nc.vector.tensor_copy(dbg[:, o + 2:o + 4], bi1[:])
                nc.vector.tensor_copy(dbg[:, o + 4:o + 6], stats2[:])
                nc.vector.tensor_copy(dbg[:, o + 6:o + 7], sc2[:])
                nc.vector.tensor_copy(dbg[:, o + 7:o + 8], bi2[:])
                nc.sync.dma_start(dbg_d[:, :], dbg[:])
                dbg2 = scrp.tile([128, 12], F32, tag="dbg2")
                nc.vector.tensor_copy(dbg2[:, 0:2], h1r[:, 0, 0:2])
                nc.vector.tensor_copy(dbg2[:, 2:4], h1r[:, 1, 0:2])
                nc.vector.tensor_copy(dbg2[:, 4:6], h2_ps[:, 0:2])
                nc.vector.tensor_copy(dbg2[:, 6:8], h2r[:, 0:2])
                nc.sync.dma_start(dbg2_d[:, :], dbg2[:])
                dbg_h = scrp.tile([128, 2 * SHARD], F32, tag="dbgh")
                for h in range(2):
                    nc.vector.tensor_copy(
                        dbg_h[:, h * SHARD:(h + 1) * SHARD], h1_ps[h][:])
                nc.sync.dma_start(dbg_h_d[:, :], dbg_h[:])

    if split_waits:
        _split_multiwaits(nc)
    return nc


_NC_CACHE = None


def _get_nc():
    global _NC_CACHE
    if _NC_CACHE is None:
        _NC_CACHE = build_program()
    return _NC_CACHE


def make_in_maps(X_sparse, X_dense, fm1_emb, bias, fm1_dense_W, fm1_dense_b,
                 emb_tables, dense_W, dense_b,
                 W1, b1, g1, beta1, W2, b2, g2, beta2, Wout, bout):
    bf16 = ml_dtypes.bfloat16
    f32 = np.float32

    g2t = np.zeros((V, ROW), dtype=bf16)
    g2t[:, 0] = fm1_emb[:, 0].astype(bf16)
    g2t[:, FEAT_OFF:FEAT_OFF + FD] = (
        np.ascontiguousarray(emb_tables.transpose(1, 0, 2)).reshape(V, FD)
        .astype(bf16)
    )

    # W1 permuted to g-order (field-major) rows, chunk-packed.
    W1p = np.ascontiguousarray(
        W1.reshape(H1, F, F, D).transpose(2, 1, 3, 0)
    ).reshape(DNN_IN, H1)
    # dense_W and dense_b in the same g-order.
    dWr = np.ascontiguousarray(
        dense_W.reshape(F, F, D, NDENSE).transpose(1, 0, 2, 3)
    ).reshape(DNN_IN, NDENSE)
    dbr = np.ascontiguousarray(
        dense_b.reshape(F, F, D).transpose(1, 0, 2)
    ).reshape(DNN_IN)

    w1k = np.zeros((N_CHUNKS, 128, H1), dtype=f32)
    dwrk = np.zeros((NDENSE + 1, N_CHUNKS * 128), dtype=bf16)
    for ci in range(N_CHUNKS):
        rows = _chunk_rows(ci)
        w1k[ci, 0:len(rows)] = W1p[rows]
        dwrk[0:NDENSE, ci * 128:ci * 128 + len(rows)] = dWr[rows].T.astype(bf16)
        dwrk[NDENSE, ci * 128:ci * 128 + len(rows)] = dbr[rows].astype(bf16)
    w1h = np.ascontiguousarray(w1k.transpose(1, 0, 2)).reshape(
        128, N_CHUNKS * H1).astype(bf16)

    w2h = np.ascontiguousarray(
        W2.T.reshape(2, 128, H2).transpose(1, 0, 2)
    ).reshape(128, H1).astype(bf16)
    wouth = Wout.reshape(H2, 1).astype(bf16) if Wout.shape == (H2, 1) else \
        Wout.T.astype(bf16)
    fm1wh = fm1_dense_W.T.astype(bf16)  # [13, 1]

    bn1gh = np.ascontiguousarray(g1.reshape(2, 128).T).astype(f32)
    bn1bh = np.ascontiguousarray(beta1.reshape(2, 128).T).astype(f32)
    bn2gh = g2.reshape(128, 1).astype(f32)
    bn2bh = beta2.reshape(128, 1).astype(f32)
    c0h = np.full((128, 1),
                  float(bias[0]) + float(fm1_dense_b[0]) + float(bout[0]),
                  dtype=f32)

    Xg = (X_sparse.astype(np.int64) +
          (np.arange(F, dtype=np.int64) * V_FIELD)[None, :]).astype(np.int32)

    in_maps = []
    for c in range(N_CORES):
        sl = slice(c * SHARD, (c + 1) * SHARD)
        xg_c = Xg[sl]                       # [512, 26]
        idx_c = np.zeros((128, NS * F), dtype=np.int32)
        for s in range(NS):
            idx_c[:, s * F:(s + 1) * F] = xg_c[s * 128:(s + 1) * 128, :]
        xdt_c = np.ones((NDENSE + 1, SHARD), dtype=bf16)
        xdt_c[0:NDENSE] = X_dense[sl].T.astype(bf16)
        in_maps.append({
            "g2": g2t, "idx": idx_c, "w1": w1h, "dwrk": dwrk, "xdt": xdt_c,
            "w2": w2h, "wout": wouth, "fm1w": fm1wh,
            "bn1g": bn1gh, "bn1b": bn1bh, "bn2g": bn2gh, "bn2b": bn2bh,
            "c0": c0h,
        })
    return in_maps


def kernel(**inputs):
    nc = _get_nc()
    in_maps = make_in_maps(**{k: np.asarray(v) for k, v in inputs.items()})
    res = run_bass_kernel_spmd(
        nc, in_maps, core_ids=list(range(N_CORES)),
        trace=bool(int(os.environ.get("DFM_TRACE", "0"))),
    )
    out = np.concatenate([res.results[c]["out"] for c in range(N_CORES)], axis=0)
    kernel.last_results = res
    return out.astype(np.float32)


# revision 22
# speedup vs baseline: 1.0263x; 1.0263x over previous
"""Trainium2 Bass kernel for nn_DeepFM_3066606649824.

Strategy (8 NeuronCores, data-parallel over batch):
  - Host: restructure the 26 FFM embedding tables [26, 208000, 16] f32 into one
    bf16 row-major table G2 [208000, 432]: col 0 = fm1_emb, cols 8:424 = the 26
    tables' rows concatenated (table-major). One gathered row then serves the
    fm1 sum, the FFM second-order products, and the DNN input.
  - Each core takes 512 batch rows and gathers its 512*26 = 13312 rows with
    indirect DMA (864B/row), issued FIELD-major so downstream compute pipelines
    under the gather window (the gather is SWDGE-issue-bound on the Pool
    engine, ~1.4us per 128-row gather => ~146us; everything else must hide
    under it).
  - fm2 via DVE scalar_tensor_tensor with an i<j access pattern on the raw
    gathered rows (half the work of the full-matrix + diagonal version);
    op (s, j) only needs fields 0..j so it runs as soon as field j lands.
  - DNN: PE-transposes g chunks to [feature, batch], the dense-path
    rd.T = relu(dWr_chunk.T @ Xd.T) is computed per chunk on PE (K=14 matmul)
    and folded during the PSUM->SBUF staging STT (relu(rd)+gT), then h1.T
    accumulates over 87 chunks on PE in bf16 with f32 PSUM. The PE stream is
    software-pipelined (transpose chunk ci, then matmul chunk ci-1).
  - BatchNorm stats are all-reduced across the 8 cores (exact); a dummy
    warmup AllReduce at t0 absorbs first-collective latency.
"""

import os
import sys

for _p in ("/opt/trn_rl_repo",):
    if _p not in sys.path and os.path.isdir(_p):
        sys.path.insert(0, _p)

import numpy as np
import ml_dtypes

from concourse import bass, mybir
import concourse.tile as tile
from concourse.vector_clock import ScopedClock
from concourse.bass_utils import run_bass_kernel_spmd
from concourse.masks import make_identity

BF16 = mybir.dt.bfloat16
F32 = mybir.dt.float32
I32 = mybir.dt.int32
AF = mybir.ActivationFunctionType
OP = mybir.AluOpType

# N_CORES only controls how many cores run (replica groups / in_maps);
# the per-core shard is fixed at BS/8. N_CORES<8 is a debug mode where only
# the first N_CORES shards are computed (BN stats then cover only those).
N_CORES = int(os.environ.get("DFM_N_CORES", "8"))
F = 26
V_FIELD = 8000
V = F * V_FIELD            # 208000
D = 16
FD = F * D                 # 416
ROW = 432                  # padded G2 row: [fm1, 7 pad, 416 feats, 8 pad]
FEAT_OFF = 8
DNN_IN = F * F * D         # 10816
H1, H2 = 256, 128
BS = 4096
SHARD = BS // 8            # 512
NS = SHARD // 128          # batch sub-tiles of 128
NDENSE = 13
EPS = 1e-5

# K-chunk map for the main matmul, ordered so every chunk is ready as soon as
# its field(s) are gathered: per tail-group t (fields 3t..3t+2): the fields'
# three full 128-row chunks each, then the packed 32-row tails chunk.
TAIL_GROUP = 3
N_TAIL = (F + TAIL_GROUP - 1) // TAIL_GROUP      # 9
CHUNKS = []  # (kind, payload): ("full", (j, piece)) | ("tail", t)
for _t in range(N_TAIL):
    for _u in range(min(TAIL_GROUP, F - TAIL_GROUP * _t)):
        for _p in range(3):
            CHUNKS.append(("full", (TAIL_GROUP * _t + _u, _p)))
    CHUNKS.append(("tail", _t))
N_CHUNKS = len(CHUNKS)     # 87


def _chunk_k(kind, payload):
    if kind == "full":
        return 128
    t = payload
    return 32 * min(TAIL_GROUP, F - TAIL_GROUP * t)


def _chunk_rows(ci):
    """Feature indices (in W1p g-order) for chunk ci's rows."""
    kind, payload = CHUNKS[ci]
    if kind == "full":
        j, p = payload
        return list(range(j * FD + 128 * p, j * FD + 128 * (p + 1)))
    t = payload
    rows = []
    for u in range(min(TAIL_GROUP, F - TAIL_GROUP * t)):
        j = TAIL_GROUP * t + u
        rows.extend(range(j * FD + 384, j * FD + FD))
    return rows


def _install_drain_split():
    """This container's walrus rejects >1 sync-wait per TPB_CTRL instruction;
    split the Tile kernel-tail drain's waits onto single-wait NOPs."""
    if getattr(tile.TileContext, "_dfm_drain_patched", False):
        return

    def _split_drain_and_barrier(self, tick_clock, wait_clock):
        collector = self.nc.sync.nop(nofuse=True)
        wait_clock.add_sem_waits(
            collector.ins, ScopedClock({None: tick_clock.global_clock})
        )
        si = collector.ins.sync_info
        waits = list(si.on_wait) if si is not None else []
        if len(waits) > 1:
            si.on_wait = waits[:1]
            for i in range(1, len(waits)):
                extra = self.nc.sync.nop(nofuse=True)
                extra.ins.sync_info = mybir.SyncInfo(
                    on_wait=[waits[i]], on_update=[]
                )
        self.nc.sync.drain()
        self.nc.all_engine_barrier()
        assert self.sems is not None
        popped = self.nc._tile_sem_poison_stack.pop()
        assert popped is self._sem_poison
        self.nc.clear_and_free_semaphores(list(self.sems.allocated().values()))
        self.nc.all_engine_barrier()

    tile.TileContext._drain_and_barrier = _split_drain_and_barrier
    tile.TileContext._dfm_drain_patched = True


def _split_multiwaits(nc, max_waits=1):
    """This walrus build also rejects >1 sync-wait on regular engine
    instructions: hoist extra waits onto single-wait NOPs just before."""
    n_split = 0
    for fn in nc.m.functions:
        for bb in fn.blocks:
            new_insts = []
            for inst in bb.instructions:
                si = getattr(inst, "sync_info", None)
                waits = list(si.on_wait) if si is not None and si.on_wait else []
                if len(waits) > max_waits:
                    keep = waits[-max_waits:]
                    for k, w in enumerate(waits[:-max_waits]):
                        nop = mybir.InstNoOp(
                            name=f"{inst.name}_w{k}",
                            engine=inst.engine,
                            sync_info=mybir.SyncInfo(
                                on_wait=[w], on_update=[]
                            ),
                            bass_nofuse=True,
                        )
                        new_insts.append(nop)
                    si.on_wait = keep
                    n_split += 1
                new_insts.append(inst)
            bb.instructions[:] = new_insts
    return n_split


def build_program(split_waits=True):
    _install_drain_split()
    nc = bass.Bass()

    g2_d = nc.declare_dram_parameter("g2", [V, ROW], BF16, isOutput=False)
    idx_d = nc.declare_dram_parameter("idx", [128, NS * F], I32, isOutput=False)
    w1_d = nc.declare_dram_parameter("w1", [128, N_CHUNKS * H1], BF16, isOutput=False)
    dwrk_d = nc.declare_dram_parameter(
        "dwrk", [NDENSE + 1, N_CHUNKS * 128], BF16, isOutput=False)
    xdt_d = nc.declare_dram_parameter("xdt", [NDENSE + 1, SHARD], BF16, isOutput=False)
    w2_d = nc.declare_dram_parameter("w2", [128, H1], BF16, isOutput=False)
    wout_d = nc.declare_dram_parameter("wout", [128, 1], BF16, isOutput=False)
    fm1w_d = nc.declare_dram_parameter("fm1w", [NDENSE, 1], BF16, isOutput=False)
    bn1g_d = nc.declare_dram_parameter("bn1g", [128, 2], F32, isOutput=False)
    bn1b_d = nc.declare_dram_parameter("bn1b", [128, 2], F32, isOutput=False)
    bn2g_d = nc.declare_dram_parameter("bn2g", [128, 1], F32, isOutput=False)
    bn2b_d = nc.declare_dram_parameter("bn2b", [128, 1], F32, isOutput=False)
    c0_d = nc.declare_dram_parameter("c0", [128, 1], F32, isOutput=False)
    out_d = nc.declare_dram_parameter("out", [SHARD, 1], F32, isOutput=True)
    debug_taps = bool(int(os.environ.get("DFM_DEBUG", "0")))
    if debug_taps:
        dbg_d = nc.declare_dram_parameter(
            "dbg", [128, NS * F + 40], F32, isOutput=True)
        dbg_h_d = nc.declare_dram_parameter(
            "dbg_h", [128, 2 * SHARD], F32, isOutput=True)
        dbg2_d = nc.declare_dram_parameter(
            "dbg2", [128, 12], F32, isOutput=True)

    groups = [list(range(N_CORES))]

    with tile.TileContext(nc) as tc:
        with (
            tc.tile_pool(name="persist", bufs=1) as persist,
            tc.tile_pool(name="gbuf", bufs=1) as gbuf,
            tc.tile_pool(name="scr", bufs=1) as scrp,
            tc.tile_pool(name="stage", bufs=6) as stagep,
            tc.tile_pool(name="rdst", bufs=8) as rdstp,
            tc.tile_pool(name="small", bufs=2) as small,
            tc.tile_pool(name="ps_h1", bufs=1, space="PSUM") as ps_h1,
            tc.tile_pool(name="ps_stage", bufs=2, space="PSUM") as ps_stage,
            tc.tile_pool(name="ps_rd", bufs=2, space="PSUM") as ps_rd,
            tc.tile_pool(name="ps_small", bufs=1, space="PSUM") as ps_small,
            tc.tile_pool(name="dram", bufs=1, space="DRAM") as dram,
        ):
            # ---- warmup AllReduce (absorbs first-collective latency) ----
            wu_in = dram.tile([128, 1], F32, tag="wui")
            wu_out = dram.tile([128, 1], F32, tag="wuo")
            nc.gpsimd.collective_compute(
                "AllReduce", OP.add, replica_groups=groups,
                ins=[wu_in.opt()], outs=[wu_out.opt()],
            )

            # ---- load constants / weights ----
            ident = persist.tile([128, 128], BF16)
            make_identity(nc, ident[:])

            idx = persist.tile([128, NS * F], I32, tag="idx")
            nc.sync.dma_start(idx[:], idx_d[:])
            xdt = persist.tile([NDENSE + 1, SHARD], BF16, tag="xdt")
            nc.sync.dma_start(xdt[:], xdt_d[:])
            dwrk = persist.tile([NDENSE + 1, N_CHUNKS * 128], BF16, tag="dwrk")
            nc.sync.dma_start(dwrk[:], dwrk_d[:])
            w1 = persist.tile([128, N_CHUNKS * H1], BF16, tag="w1")
            # w1 is 5.7MB; load in 4 slices so the first chunks' weights
            # arrive quickly and spread across queues.
            for q in range(4):
                sl = slice(q * (N_CHUNKS * H1 // 4), (q + 1) * (N_CHUNKS * H1 // 4))
                nc.sync.dma_start(w1[:, sl], w1_d[:, sl])
            w2 = persist.tile([128, H1], BF16, tag="w2")
            nc.sync.dma_start(w2[:], w2_d[:])
            wout = persist.tile([128, 1], BF16, tag="wout")
            nc.sync.dma_start(wout[:], wout_d[:])
            fm1w = persist.tile([NDENSE, 1], BF16, tag="fm1w")
            nc.sync.dma_start(fm1w[:], fm1w_d[:])
            bn1g = persist.tile([128, 2], F32, tag="bn1g")
            nc.sync.dma_start(bn1g[:], bn1g_d[:])
            bn1b = persist.tile([128, 2], F32, tag="bn1b")
            nc.sync.dma_start(bn1b[:], bn1b_d[:])
            bn2g = persist.tile([128, 1], F32, tag="bn2g")
            nc.sync.dma_start(bn2g[:], bn2g_d[:])
            bn2b = persist.tile([128, 1], F32, tag="bn2b")
            nc.sync.dma_start(bn2b[:], bn2b_d[:])
            c0 = persist.tile([128, 1], F32, tag="c0")
            nc.sync.dma_start(c0[:], c0_d[:])

            g = [gbuf.tile([128, F * ROW], BF16, tag=f"g{s}", name=f"g{s}")
                 for s in range(NS)]

            # head: the dense fm1 part is independent of the gather; run early.
            head_ps = ps_small.tile([128, 2 * NS], F32, tag="heads")
            for s in range(NS):
                nc.tensor.matmul(
                    head_ps[:, NS + s:NS + s + 1],
                    lhsT=xdt[0:NDENSE, s * 128:(s + 1) * 128],
                    rhs=fm1w[:],
                    start=True, stop=True,
                )

            # ---- pipelined gather + fm2 + transpose/fold/matmul ----
            h1_ps = [
                ps_h1.tile([128, SHARD], F32, tag=f"h1_{h}", name=f"h1_ps{h}")
                for h in range(2)
            ]
            Spart = small.tile([128, NS * F], F32, tag="fm2Spart")
            scr = scrp.tile([128, FD], BF16, tag="fm2scr")
            part0 = None

            pend = []          # chunks transposed+folded but not yet matmuled
            rdq = {}           # ci -> (rdr tile, kc), rd pipeline lookahead
            RD_LOOKAHEAD = 6
            PEND_DEPTH = 3

            def emit_rd(ci):
                """rd (dense-path) pipeline: PE K=14 matmul + Act relu to
                SBUF bf16. Independent of the gather, so it runs ahead."""
                kind, payload = CHUNKS[ci]
                kc = _chunk_k(kind, payload)
                rd_ps = ps_rd.tile([128, SHARD], F32, tag="rd")
                nc.tensor.matmul(
                    rd_ps[0:kc, :],
                    lhsT=dwrk[:, ci * 128: ci * 128 + kc],
                    rhs=xdt[:, :],
                    start=True, stop=True,
                )
                rdr = rdstp.tile([128, SHARD], BF16, tag="rdr")
                nc.scalar.activation(
                    out=rdr[0:kc, :], in_=rd_ps[0:kc, :], func=AF.Relu,
                )
                rdq[ci] = (rdr, kc)

            def emit_mm(ci, rhs, kc):
                nc.tensor.matmul(
                    h1_ps[0][:],
                    lhsT=w1[0:kc, ci * H1 + 0 * 128: ci * H1 + 1 * 128],
                    rhs=rhs[0:kc, :],
                    start=(ci == 0), stop=(ci == N_CHUNKS - 1),
                )
                nc.tensor.matmul(
                    h1_ps[1][:],
                    lhsT=w1[0:kc, ci * H1 + 1 * 128: ci * H1 + 2 * 128],
                    rhs=rhs[0:kc, :],
                    start=(ci == 0), stop=(ci == N_CHUNKS - 1),
                )

            def emit_chunk(ci):
                """Transpose chunk ci into PSUM, fold with the (pre-computed)
                rd into SBUF rhs; defer its h1 matmuls by PEND_DEPTH chunks
                (PE pipeline skew)."""
                if ci + RD_LOOKAHEAD < N_CHUNKS:
                    emit_rd(ci + RD_LOOKAHEAD)
                kind, payload = CHUNKS[ci]
                kc = _chunk_k(kind, payload)
                st_ps = ps_stage.tile([128, SHARD], BF16, tag="st")
                if kind == "full":
                    j, p = payload
                    off = j * ROW + FEAT_OFF + 128 * p
                    for s in range(NS):
                        nc.tensor.transpose(
                            out=st_ps[0:128, s * 128:(s + 1) * 128],
                            in_=g[s][:, off:off + 128],
                            identity=ident[:],
                        )
                else:
                    t = payload
                    for u in range(min(TAIL_GROUP, F - TAIL_GROUP * t)):
                        j = TAIL_GROUP * t + u
                        off = j * ROW + FEAT_OFF + 384
                        for s in range(NS):
                            nc.tensor.transpose(
                                out=st_ps[32 * u:32 * (u + 1),
                                          s * 128:(s + 1) * 128],
                                in_=g[s][:, off:off + 32],
                                identity=ident[:],
                            )
                # rhs = gT + relu(rd); rdr was staged ahead by the rd pipeline
                rdr, kc2 = rdq.pop(ci)
                assert kc2 == kc
                nc.tensor.ldweights(ident[:])  # HAM keep-warm, no output
                rhs = stagep.tile([128, SHARD], BF16, tag="rhs")
                nc.vector.tensor_tensor(
                    out=rhs[0:kc, :], in0=st_ps[0:kc, :], in1=rdr[0:kc, :],
                    op=OP.add,
                )
                pend.append((ci, rhs, kc))
                if len(pend) > PEND_DEPTH:
                    emit_mm(*pend.pop(0))

            for ci0 in range(min(RD_LOOKAHEAD, N_CHUNKS)):
                emit_rd(ci0)

            ci_iter = 0
            for t in range(N_TAIL):
                nf = min(TAIL_GROUP, F - TAIL_GROUP * t)
                # all the group's gathers first (writers must be emitted
                # before any reader for Tile dep tracking)
                for u in range(nf):
                    j = TAIL_GROUP * t + u
                    for s in range(NS):
                        nc.gpsimd.indirect_dma_start(
                            out=g[s][:, j * ROW:(j + 1) * ROW],
                            out_offset=None,
                            in_=g2_d[:, :],
                            in_offset=bass.IndirectOffsetOnAxis(
                                ap=idx[:, s * F + j:s * F + j + 1], axis=0
                            ),
                        )
                if part0 is None:
                    part0 = list(g[0][:].ap[0])
                # fm2 cross terms for field j against fields i<j:
                #   Spart[:, j] = sum_{i<j,d} g[p, j, 16i+d] * g[p, i, 16j+d]
                for u in range(nf):
                    j = TAIL_GROUP * t + u
                    if j == 0:
                        continue
                    for s in range(NS):
                        g_s = g[s]
                        in0 = bass.AP(
                            g_s[:].tensor,
                            g_s[:].offset + j * ROW + FEAT_OFF,
                            [part0, [D, j], [1, D]],
                        )
                        in1 = bass.AP(
                            g_s[:].tensor,
                            g_s[:].offset + FEAT_OFF + D * j,
                            [part0, [ROW, j], [1, D]],
                        )
                        nc.vector.scalar_tensor_tensor(
                            out=scr[:, 0:j * D].rearrange(
                                "p (i d) -> p i d", d=D),
                            in0=in0, scalar=1.0, in1=in1,
                            op0=OP.mult, op1=OP.mult,
                            accum_out=Spart[:, s * F + j:s * F + j + 1],
                        )
                for _ in range(3 * nf + 1):
                    emit_chunk(ci_iter)
                    ci_iter += 1
            assert ci_iter == N_CHUNKS
            while pend:
                emit_mm(*pend.pop(0))

            # ---- fm1 (embedding part) and fm2 reduction ----
            fm1e = small.tile([128, NS], F32, tag="fm1e")
            S_acc = small.tile([128, NS], F32, tag="fm2S")
            for s in range(NS):
                fm1_ap = g[s][:].rearrange("p (j e) -> p j e", j=F)[:, :, 0]
                nc.vector.tensor_reduce(
                    out=fm1e[:, s:s + 1], in_=fm1_ap,
                    axis=mybir.AxisListType.X, op=OP.add,
                )
                nc.vector.tensor_reduce(
                    out=S_acc[:, s:s + 1],
                    in_=Spart[:, s * F + 1:(s + 1) * F],
                    axis=mybir.AxisListType.X, op=OP.add,
                )

            # ---- BN1 stats + allreduce ----
            stats1 = small.tile([128, 4], F32, tag="stats1")
            sq_scr = scrp.tile([128, SHARD], F32, tag="sq")
            for h in range(2):
                nc.vector.tensor_reduce(
                    out=stats1[:, h:h + 1], in_=h1_ps[h][:],
                    axis=mybir.AxisListType.X, op=OP.add,
                )
                nc.scalar.activation(
                    out=sq_scr[:], in_=h1_ps[h][:], func=AF.Square,
                    accum_out=stats1[:, 2 + h:3 + h],
                )
            b1_in = dram.tile([128, 4], F32, tag="b1i")
            b1_out = dram.tile([128, 4], F32, tag="b1o")
            nc.sync.dma_start(b1_in[:], stats1[:])
            nc.gpsimd.collective_compute(
                "AllReduce", OP.add, replica_groups=groups,
                ins=[b1_in.opt()], outs=[b1_out.opt()],
            )
            stats1g = small.tile([128, 4], F32, tag="stats1g")
            nc.sync.dma_start(stats1g[:], b1_out[:])

            def bn_scale_bias(statsg, col_s, col_q, gamma, beta, ncols):
                # returns (scale, bias) [128, ncols] f32
                mean = small.tile([128, ncols], F32, tag="bn_mean")
                var = small.tile([128, ncols], F32, tag="bn_var")
                scale = small.tile([128, ncols], F32, tag="bn_scale")
                bias = small.tile([128, ncols], F32, tag="bn_bias")
                tmp = small.tile([128, ncols], F32, tag="bn_tmp")
                nc.vector.tensor_scalar_mul(
                    mean[:], statsg[:, col_s:col_s + ncols], 1.0 / BS)
                nc.vector.tensor_scalar_mul(
                    var[:], statsg[:, col_q:col_q + ncols], 1.0 / BS)
                nc.vector.tensor_tensor(
                    out=tmp[:], in0=mean[:], in1=mean[:], op=OP.mult)
                nc.vector.tensor_tensor(
                    out=var[:], in0=var[:], in1=tmp[:], op=OP.subtract)
                nc.vector.tensor_scalar_add(var[:], var[:], EPS)
                nc.vector.reciprocal(tmp[:], var[:])
                nc.scalar.activation(out=tmp[:], in_=tmp[:], func=AF.Sqrt)
                nc.vector.tensor_tensor(
                    out=scale[:], in0=gamma[:], in1=tmp[:], op=OP.mult)
                nc.vector.tensor_tensor(
                    out=tmp[:], in0=mean[:], in1=scale[:], op=OP.mult)
                nc.vector.tensor_tensor(
                    out=bias[:], in0=beta[:], in1=tmp[:], op=OP.subtract)
                return scale, bias

            sc1, bi1 = bn_scale_bias(stats1g, 0, 2, bn1g, bn1b, 2)
            h1r = persist.tile([128, 2, SHARD], BF16, tag="h1r")
            for h in range(2):
                nc.scalar.activation(
                    out=h1r[:, h, :], in_=h1_ps[h][:], func=AF.Relu,
                    bias=bi1[:, h:h + 1], scale=sc1[:, h:h + 1],
                )

            # ---- layer 2 ----
            h2_ps = ps_small.tile([128, SHARD], F32, tag="h2")
            for h in range(2):
                nc.tensor.matmul(
                    h2_ps[:],
                    lhsT=w2[:, h * 128:(h + 1) * 128],
                    rhs=h1r[:, h, :],
                    start=(h == 0), stop=(h == 1),
                )
            stats2 = small.tile([128, 2], F32, tag="stats2")
            nc.vector.tensor_reduce(
                out=stats2[:, 0:1], in_=h2_ps[:],
                axis=mybir.AxisListType.X, op=OP.add,
            )
            sq_scr2 = scrp.tile([128, SHARD], F32, tag="sq")
            nc.scalar.activation(
                out=sq_scr2[:], in_=h2_ps[:], func=AF.Square,
                accum_out=stats2[:, 1:2],
            )
            b2_in = dram.tile([128, 2], F32, tag="b2i")
            b2_out = dram.tile([128, 2], F32, tag="b2o")
            nc.sync.dma_start(b2_in[:], stats2[:])
            nc.gpsimd.collective_compute(
                "AllReduce", OP.add, replica_groups=groups,
                ins=[b2_in.opt()], outs=[b2_out.opt()],
            )
            stats2g = small.tile([128, 2], F32, tag="stats2g")
            nc.sync.dma_start(stats2g[:], b2_out[:])
            sc2, bi2 = bn_scale_bias(stats2g, 0, 1, bn2g, bn2b, 1)
            h2r = persist.tile([128, SHARD], BF16, tag="h2r")
            nc.scalar.activation(
                out=h2r[:], in_=h2_ps[:], func=AF.Relu,
                bias=bi2[:, 0:1], scale=sc2[:, 0:1],
            )

            # ---- heads ----
            for s in range(NS):
                nc.tensor.matmul(
                    head_ps[:, s:s + 1],
                    lhsT=h2r[:, s * 128:(s + 1) * 128],
                    rhs=wout[:],
                    start=True, stop=True,
                )

            tot = small.tile([128, NS], F32, tag="tot")
            res = small.tile([128, NS], F32, tag="res")
            nc.vector.tensor_tensor(
                out=tot[:], in0=fm1e[:], in1=head_ps[:, 0:NS], op=OP.add)
            nc.vector.tensor_tensor(
                out=tot[:], in0=tot[:], in1=head_ps[:, NS:2 * NS], op=OP.add)
            nc.vector.tensor_tensor(
                out=tot[:], in0=tot[:], in1=S_acc[:], op=OP.add)
            nc.scalar.activation(
                out=res[:], in_=tot[:], func=AF.Sigmoid,
                bias=c0[:, 0:1], scale=1.0,
            )
            out_ap = out_d[:, :].rearrange("(s p) o -> p (s o)", p=128)
            nc.sync.dma_start(out_ap, res[:])

            if debug_taps:
                dbg = scrp.tile([128, NS * F + 40], F32, tag="dbg")
                nc.vector.tensor_copy(dbg[:, 0:NS * F], Spart[:])
                nc.vector.tensor_copy(
                    dbg[:, NS * F:NS * F + NS], fm1e[:])
                nc.vector.tensor_copy(
                    dbg[:, NS * F + NS:NS * F + 2 * NS], S_acc[:])
                nc.vector.tensor_copy(
                    dbg[:, NS * F + 8:NS * F + 12], stats1[:])
                nc.vector.tensor_copy(
                    dbg[:, NS * F + 12:NS * F + 16], stats1g[:])
                nc.vector.tensor_copy(
                    dbg[:, NS * F + 16:NS * F + 16 + 2 * NS],
                    head_ps[:, 0:2 * NS])
                o = NS * F + 24
                nc.vector.tensor_copy(dbg[:, o:o + 2], sc1[:])
                nc.vector.tensor_copy(dbg[:, o + 2:o + 4], bi1[:])
                nc.vector.tensor_copy(dbg[:, o + 4:o + 6], stats2[:])
                nc.vector.tensor_copy(dbg[:, o + 6:o + 7], sc2[:])
                nc.vector.tensor_copy(dbg[:, o + 7:o + 8], bi2[:])
                nc.sync.dma_start(dbg_d[:, :], dbg[:])
                dbg2 = scrp.tile([128, 12], F32, tag="dbg2")
                nc.vector.tensor_copy(dbg2[:, 0:2], h1r[:, 0, 0:2])
                nc.vector.tensor_copy(dbg2[:, 2:4], h1r[:, 1, 0:2])
                nc.vector.tensor_copy(dbg2[:, 4:6], h2_ps[:, 0:2])
                nc.vector.tensor_copy(dbg2[:, 6:8], h2r[:, 0:2])
                nc.sync.dma_start(dbg2_d[:, :], dbg2[:])
                dbg_h = scrp.tile([128, 2 * SHARD], F32, tag="dbgh")
                for h in range(2):
                    nc.vector.tensor_copy(
                        dbg_h[:, h * SHARD:(h + 1) * SHARD], h1_ps[h][:])
                nc.sync.dma_start(dbg_h_d[:, :], dbg_h[:])

    if split_waits:
        _split_multiwaits(nc)
    return nc


_NC_CACHE = None


def _get_nc():
    global _NC_CACHE
    if _NC_CACHE is None:
        _NC_CACHE = build_program()
    return _NC_CACHE


def make_in_maps(X_sparse, X_dense, fm1_emb, bias, fm1_dense_W, fm1_dense_b,
                 emb_tables, dense_W, dense_b,
                 W1, b1, g1, beta1, W2, b2, g2, beta2, Wout, bout):
    bf16 = ml_dtypes.bfloat16
    f32 = np.float32

    g2t = np.zeros((V, ROW), dtype=bf16)
    g2t[:, 0] = fm1_emb[:, 0].astype(bf16)
    g2t[:, FEAT_OFF:FEAT_OFF + FD] = (
        np.ascontiguousarray(emb_tables.transpose(1, 0, 2)).reshape(V, FD)
        .astype(bf16)
    )

    # W1 permuted to g-order (field-major) rows, chunk-packed.
    W1p = np.ascontiguousarray(
        W1.reshape(H1, F, F, D).transpose(2, 1, 3, 0)
    ).reshape(DNN_IN, H1)
    # dense_W and dense_b in the same g-order.
    dWr = np.ascontiguousarray(
        dense_W.reshape(F, F, D, NDENSE).transpose(1, 0, 2, 3)
    ).reshape(DNN_IN, NDENSE)
    dbr = np.ascontiguousarray(
        dense_b.reshape(F, F, D).transpose(1, 0, 2)
    ).reshape(DNN_IN)

    w1k = np.zeros((N_CHUNKS, 128, H1), dtype=f32)
    dwrk = np.zeros((NDENSE + 1, N_CHUNKS * 128), dtype=bf16)
    for ci in range(N_CHUNKS):
        rows = _chunk_rows(ci)
        w1k[ci, 0:len(rows)] = W1p[rows]
        dwrk[0:NDENSE, ci * 128:ci * 128 + len(rows)] = dWr[rows].T.astype(bf16)
        dwrk[NDENSE, ci * 128:ci * 128 + len(rows)] = dbr[rows].astype(bf16)
    w1h = np.ascontiguousarray(w1k.transpose(1, 0, 2)).reshape(
        128, N_CHUNKS * H1).astype(bf16)

    w2h = np.ascontiguousarray(
        W2.T.reshape(2, 128, H2).transpose(1, 0, 2)
    ).reshape(128, H1).astype(bf16)
    wouth = Wout.reshape(H2, 1).astype(bf16) if Wout.shape == (H2, 1) else \
        Wout.T.astype(bf16)
    fm1wh = fm1_dense_W.T.astype(bf16)  # [13, 1]

    bn1gh = np.ascontiguousarray(g1.reshape(2, 128).T).astype(f32)
    bn1bh = np.ascontiguousarray(beta1.reshape(2, 128).T).astype(f32)
    bn2gh = g2.reshape(128, 1).astype(f32)
    bn2bh = beta2.reshape(128, 1).astype(f32)
    c0h = np.full((128, 1),
                  float(bias[0]) + float(fm1_dense_b[0]) + float(bout[0]),
                  dtype=f32)

    Xg = (X_sparse.astype(np.int64) +
          (np.arange(F, dtype=np.int64) * V_FIELD)[None, :]).astype(np.int32)

    in_maps = []
    for c in range(N_CORES):
        sl = slice(c * SHARD, (c + 1) * SHARD)
        xg_c = Xg[sl]                       # [512, 26]
        idx_c = np.zeros((128, NS * F), dtype=np.int32)
        for s in range(NS):
            idx_c[:, s * F:(s + 1) * F] = xg_c[s * 128:(s + 1) * 128, :]
        xdt_c = np.ones((NDENSE + 1, SHARD), dtype=bf16)
        xdt_c[0:NDENSE] = X_dense[sl].T.astype(bf16)
        in_maps.append({
            "g2": g2t, "idx": idx_c, "w1": w1h, "dwrk": dwrk, "xdt": xdt_c,
            "w2": w2h, "wout": wouth, "fm1w": fm1wh,
            "bn1g": bn1gh, "bn1b": bn1bh, "bn2g": bn2gh, "bn2b": bn2bh,
            "c0": c0h,
        })
    return in_maps


def kernel(**inputs):
    nc = _get_nc()
    in_maps = make_in_maps(**{k: np.asarray(v) for k, v in inputs.items()})
    res = run_bass_kernel_spmd(
        nc, in_maps, core_ids=list(range(N_CORES)),
        trace=bool(int(os.environ.get("DFM_TRACE", "0"))),
    )
    out = np.concatenate([res.results[c]["out"] for c in range(N_CORES)], axis=0)
    kernel.last_results = res
    return out.astype(np.float32)


# revision 23
# speedup vs baseline: 1.1175x; 1.0889x over previous
"""Trainium2 Bass kernel for nn_DeepFM_3066606649824.

Strategy (8 NeuronCores, data-parallel over batch):
  - Host: restructure the 26 FFM embedding tables [26, 208000, 16] f32 into one
    bf16 row-major table G2 [208000, 432]: col 0 = fm1_emb, cols 8:424 = the 26
    tables' rows concatenated (table-major). One gathered row then serves the
    fm1 sum, the FFM second-order products, and the DNN input.
  - Each core takes 512 batch rows and gathers its 512*26 = 13312 rows with
    indirect DMA (864B/row), issued FIELD-major so downstream compute pipelines
    under the gather window (the gather is SWDGE-issue-bound on the Pool
    engine, ~1.4us per 128-row gather => ~146us; everything else must hide
    under it).
  - fm2 via DVE scalar_tensor_tensor with an i<j access pattern on the raw
    gathered rows (half the work of the full-matrix + diagonal version);
    op (s, j) only needs fields 0..j so it runs as soon as field j lands.
  - DNN: PE-transposes g chunks to [feature, batch], the dense-path
    rd.T = relu(dWr_chunk.T @ Xd.T) is computed per chunk on PE (K=14 matmul)
    and folded during the PSUM->SBUF staging STT (relu(rd)+gT), then h1.T
    accumulates over 87 chunks on PE in bf16 with f32 PSUM. The PE stream is
    software-pipelined (transpose chunk ci, then matmul chunk ci-1).
  - BatchNorm stats are all-reduced across the 8 cores (exact); a dummy
    warmup AllReduce at t0 absorbs first-collective latency.
"""

import os
import sys

for _p in ("/opt/trn_rl_repo",):
    if _p not in sys.path and os.path.isdir(_p):
        sys.path.insert(0, _p)

import numpy as np
import ml_dtypes

from concourse import bass, mybir
import concourse.tile as tile
from concourse.vector_clock import ScopedClock
from concourse.bass_utils import run_bass_kernel_spmd
from concourse.masks import make_identity

BF16 = mybir.dt.bfloat16
F32 = mybir.dt.float32
I32 = mybir.dt.int32
AF = mybir.ActivationFunctionType
OP = mybir.AluOpType

# N_CORES only controls how many cores run (replica groups / in_maps);
# the per-core shard is fixed at BS/8. N_CORES<8 is a debug mode where only
# the first N_CORES shards are computed (BN stats then cover only those).
N_CORES = int(os.environ.get("DFM_N_CORES", "8"))
F = 26
V_FIELD = 8000
V = F * V_FIELD            # 208000
D = 16
FD = F * D                 # 416
ROW = 432                  # padded G2 row: [fm1, 7 pad, 416 feats, 8 pad]
FEAT_OFF = 8
DNN_IN = F * F * D         # 10816
H1, H2 = 256, 128
BS = 4096
SHARD = BS // 8            # 512
NS = SHARD // 128          # batch sub-tiles of 128
NDENSE = 13
EPS = 1e-5

# K-chunk map for the main matmul, ordered so every chunk is ready as soon as
# its field(s) are gathered: per tail-group t (fields 3t..3t+2): the fields'
# three full 128-row chunks each, then the packed 32-row tails chunk.
TAIL_GROUP = 3
N_TAIL = (F + TAIL_GROUP - 1) // TAIL_GROUP      # 9
CHUNKS = []  # (kind, payload): ("full", (j, piece)) | ("tail", t)
for _t in range(N_TAIL):
    for _u in range(min(TAIL_GROUP, F - TAIL_GROUP * _t)):
        for _p in range(3):
            CHUNKS.append(("full", (TAIL_GROUP * _t + _u, _p)))
    CHUNKS.append(("tail", _t))
N_CHUNKS = len(CHUNKS)     # 87


def _chunk_k(kind, payload):
    if kind == "full":
        return 128
    t = payload
    return 32 * min(TAIL_GROUP, F - TAIL_GROUP * t)


def _chunk_rows(ci):
    """Feature indices (in W1p g-order) for chunk ci's rows."""
    kind, payload = CHUNKS[ci]
    if kind == "full":
        j, p = payload
        return list(range(j * FD + 128 * p, j * FD + 128 * (p + 1)))
    t = payload
    rows = []
    for u in range(min(TAIL_GROUP, F - TAIL_GROUP * t)):
        j = TAIL_GROUP * t + u
        rows.extend(range(j * FD + 384, j * FD + FD))
    return rows


def _install_drain_split():
    """This container's walrus rejects >1 sync-wait per TPB_CTRL instruction;
    split the Tile kernel-tail drain's waits onto single-wait NOPs."""
    if getattr(tile.TileContext, "_dfm_drain_patched", False):
        return

    def _split_drain_and_barrier(self, tick_clock, wait_clock):
        collector = self.nc.sync.nop(nofuse=True)
        wait_clock.add_sem_waits(
            collector.ins, ScopedClock({None: tick_clock.global_clock})
        )
        si = collector.ins.sync_info
        waits = list(si.on_wait) if si is not None else []
        if len(waits) > 1:
            si.on_wait = waits[:1]
            for i in range(1, len(waits)):
                extra = self.nc.sync.nop(nofuse=True)
                extra.ins.sync_info = mybir.SyncInfo(
                    on_wait=[waits[i]], on_update=[]
                )
        self.nc.sync.drain()
        self.nc.all_engine_barrier()
        assert self.sems is not None
        popped = self.nc._tile_sem_poison_stack.pop()
        assert popped is self._sem_poison
        self.nc.clear_and_free_semaphores(list(self.sems.allocated().values()))
        self.nc.all_engine_barrier()

    tile.TileContext._drain_and_barrier = _split_drain_and_barrier
    tile.TileContext._dfm_drain_patched = True


def _split_multiwaits(nc, max_waits=1):
    """This walrus build also rejects >1 sync-wait on regular engine
    instructions: hoist extra waits onto single-wait NOPs just before."""
    n_split = 0
    for fn in nc.m.functions:
        for bb in fn.blocks:
            new_insts = []
            for inst in bb.instructions:
                si = getattr(inst, "sync_info", None)
                waits = list(si.on_wait) if si is not None and si.on_wait else []
                if len(waits) > max_waits:
                    keep = waits[-max_waits:]
                    for k, w in enumerate(waits[:-max_waits]):
                        nop = mybir.InstNoOp(
                            name=f"{inst.name}_w{k}",
                            engine=inst.engine,
                            sync_info=mybir.SyncInfo(
                                on_wait=[w], on_update=[]
                            ),
                            bass_nofuse=True,
                        )
                        new_insts.append(nop)
                    si.on_wait = keep
                    n_split += 1
                new_insts.append(inst)
            bb.instructions[:] = new_insts
    return n_split


def build_program(split_waits=True):
    _install_drain_split()
    nc = bass.Bass()

    g2_d = nc.declare_dram_parameter("g2", [V, ROW], BF16, isOutput=False)
    idx_d = nc.declare_dram_parameter("idx", [128, NS * F], I32, isOutput=False)
    w1_d = nc.declare_dram_parameter("w1", [128, N_CHUNKS * H1], BF16, isOutput=False)
    dwrk_d = nc.declare_dram_parameter(
        "dwrk", [NDENSE + 1, N_CHUNKS * 128], BF16, isOutput=False)
    xdt_d = nc.declare_dram_parameter("xdt", [NDENSE + 1, SHARD], BF16, isOutput=False)
    w2_d = nc.declare_dram_parameter("w2", [128, H1], BF16, isOutput=False)
    wout_d = nc.declare_dram_parameter("wout", [128, 1], BF16, isOutput=False)
    fm1w_d = nc.declare_dram_parameter("fm1w", [NDENSE, 1], BF16, isOutput=False)
    bn1g_d = nc.declare_dram_parameter("bn1g", [128, 2], F32, isOutput=False)
    bn1b_d = nc.declare_dram_parameter("bn1b", [128, 2], F32, isOutput=False)
    bn2g_d = nc.declare_dram_parameter("bn2g", [128, 1], F32, isOutput=False)
    bn2b_d = nc.declare_dram_parameter("bn2b", [128, 1], F32, isOutput=False)
    c0_d = nc.declare_dram_parameter("c0", [128, 1], F32, isOutput=False)
    out_d = nc.declare_dram_parameter("out", [SHARD, 1], F32, isOutput=True)
    debug_taps = bool(int(os.environ.get("DFM_DEBUG", "0")))
    if debug_taps:
        dbg_d = nc.declare_dram_parameter(
            "dbg", [128, NS * F + 40], F32, isOutput=True)
        dbg_h_d = nc.declare_dram_parameter(
            "dbg_h", [128, 2 * SHARD], F32, isOutput=True)
        dbg2_d = nc.declare_dram_parameter(
            "dbg2", [128, 12], F32, isOutput=True)

    groups = [list(range(N_CORES))]

    with tile.TileContext(nc) as tc:
        with (
            tc.tile_pool(name="persist", bufs=1) as persist,
            tc.tile_pool(name="gbuf", bufs=1) as gbuf,
            tc.tile_pool(name="scr", bufs=1) as scrp,
            tc.tile_pool(name="stage", bufs=6) as stagep,
            tc.tile_pool(name="rdst", bufs=25) as rdstp,
            tc.tile_pool(name="small", bufs=2) as small,
            tc.tile_pool(name="ps_h1", bufs=1, space="PSUM") as ps_h1,
            tc.tile_pool(name="ps_stage", bufs=2, space="PSUM") as ps_stage,
            tc.tile_pool(name="ps_rd", bufs=2, space="PSUM") as ps_rd,
            tc.tile_pool(name="ps_small", bufs=1, space="PSUM") as ps_small,
            tc.tile_pool(name="dram", bufs=1, space="DRAM") as dram,
        ):
            # ---- warmup AllReduce (absorbs first-collective latency) ----
            wu_in = dram.tile([128, 1], F32, tag="wui")
            wu_out = dram.tile([128, 1], F32, tag="wuo")
            nc.gpsimd.collective_compute(
                "AllReduce", OP.add, replica_groups=groups,
                ins=[wu_in.opt()], outs=[wu_out.opt()],
            )

            # ---- load constants / weights (idx first: it gates gathers) ----
            idx = persist.tile([128, NS * F], I32, tag="idx")
            nc.sync.dma_start(idx[:], idx_d[:])
            ident = persist.tile([128, 128], BF16)
            make_identity(nc, ident[:])

            xdt = persist.tile([NDENSE + 1, SHARD], BF16, tag="xdt")
            nc.sync.dma_start(xdt[:], xdt_d[:])
            dwrk = persist.tile([NDENSE + 1, N_CHUNKS * 128], BF16, tag="dwrk")
            nc.sync.dma_start(dwrk[:], dwrk_d[:])
            w1 = persist.tile([128, N_CHUNKS * H1], BF16, tag="w1")
            # w1 is 5.7MB; load in 4 slices so the first chunks' weights
            # arrive quickly and spread across queues.
            for q in range(4):
                sl = slice(q * (N_CHUNKS * H1 // 4), (q + 1) * (N_CHUNKS * H1 // 4))
                nc.sync.dma_start(w1[:, sl], w1_d[:, sl])
            w2 = persist.tile([128, H1], BF16, tag="w2")
            nc.sync.dma_start(w2[:], w2_d[:])
            wout = persist.tile([128, 1], BF16, tag="wout")
            nc.sync.dma_start(wout[:], wout_d[:])
            fm1w = persist.tile([NDENSE, 1], BF16, tag="fm1w")
            nc.sync.dma_start(fm1w[:], fm1w_d[:])
            bn1g = persist.tile([128, 2], F32, tag="bn1g")
            nc.sync.dma_start(bn1g[:], bn1g_d[:])
            bn1b = persist.tile([128, 2], F32, tag="bn1b")
            nc.sync.dma_start(bn1b[:], bn1b_d[:])
            bn2g = persist.tile([128, 1], F32, tag="bn2g")
            nc.sync.dma_start(bn2g[:], bn2g_d[:])
            bn2b = persist.tile([128, 1], F32, tag="bn2b")
            nc.sync.dma_start(bn2b[:], bn2b_d[:])
            c0 = persist.tile([128, 1], F32, tag="c0")
            nc.sync.dma_start(c0[:], c0_d[:])

            g = [gbuf.tile([128, F * ROW], BF16, tag=f"g{s}", name=f"g{s}")
                 for s in range(NS)]

            # head: the dense fm1 part is independent of the gather; run early.
            head_ps = ps_small.tile([128, 2 * NS], F32, tag="heads")
            for s in range(NS):
                nc.tensor.matmul(
                    head_ps[:, NS + s:NS + s + 1],
                    lhsT=xdt[0:NDENSE, s * 128:(s + 1) * 128],
                    rhs=fm1w[:],
                    start=True, stop=True,
                )

            # ---- pipelined gather + fm2 + transpose/fold/matmul ----
            h1_ps = [
                ps_h1.tile([128, SHARD], F32, tag=f"h1_{h}", name=f"h1_ps{h}")
                for h in range(2)
            ]
            Spart = small.tile([128, NS * F], F32, tag="fm2Spart")
            scr = scrp.tile([128, FD], BF16, tag="fm2scr")
            part0 = None

            pend = []          # chunks transposed+folded but not yet matmuled
            rdq = {}           # ci -> (rdr tile, kc), rd pipeline lookahead
            RD_LOOKAHEAD = 24
            PEND_DEPTH = 3

            def emit_rd(ci):
                """rd (dense-path) pipeline: PE K=14 matmul + Act relu to
                SBUF bf16. Independent of the gather, so it runs ahead."""
                kind, payload = CHUNKS[ci]
                kc = _chunk_k(kind, payload)
                rd_ps = ps_rd.tile([128, SHARD], F32, tag="rd")
                nc.tensor.matmul(
                    rd_ps[0:kc, :],
                    lhsT=dwrk[:, ci * 128: ci * 128 + kc],
                    rhs=xdt[:, :],
                    start=True, stop=True,
                )
                rdr = rdstp.tile([128, SHARD], BF16, tag="rdr")
                nc.scalar.activation(
                    out=rdr[0:kc, :], in_=rd_ps[0:kc, :], func=AF.Relu,
                )
                rdq[ci] = (rdr, kc)

            def emit_mm(ci, rhs, kc):
                nc.tensor.matmul(
                    h1_ps[0][:],
                    lhsT=w1[0:kc, ci * H1 + 0 * 128: ci * H1 + 1 * 128],
                    rhs=rhs[0:kc, :],
                    start=(ci == 0), stop=(ci == N_CHUNKS - 1),
                )
                nc.tensor.matmul(
                    h1_ps[1][:],
                    lhsT=w1[0:kc, ci * H1 + 1 * 128: ci * H1 + 2 * 128],
                    rhs=rhs[0:kc, :],
                    start=(ci == 0), stop=(ci == N_CHUNKS - 1),
                )

            def emit_chunk(ci):
                """Transpose chunk ci into PSUM, fold with the (pre-computed)
                rd into SBUF rhs; defer its h1 matmuls by PEND_DEPTH chunks
                (PE pipeline skew)."""
                if ci + RD_LOOKAHEAD < N_CHUNKS:
                    emit_rd(ci + RD_LOOKAHEAD)
                kind, payload = CHUNKS[ci]
                kc = _chunk_k(kind, payload)
                st_ps = ps_stage.tile([128, SHARD], BF16, tag="st")
                if kind == "full":
                    j, p = payload
                    off = j * ROW + FEAT_OFF + 128 * p
                    for s in range(NS):
                        nc.tensor.transpose(
                            out=st_ps[0:128, s * 128:(s + 1) * 128],
                            in_=g[s][:, off:off + 128],
                            identity=ident[:],
                        )
                else:
                    t = payload
                    for u in range(min(TAIL_GROUP, F - TAIL_GROUP * t)):
                        j = TAIL_GROUP * t + u
                        off = j * ROW + FEAT_OFF + 384
                        for s in range(NS):
                            nc.tensor.transpose(
                                out=st_ps[32 * u:32 * (u + 1),
                                          s * 128:(s + 1) * 128],
                                in_=g[s][:, off:off + 32],
                                identity=ident[:],
                            )
                # rhs = gT + relu(rd); rdr was staged ahead by the rd pipeline
                rdr, kc2 = rdq.pop(ci)
                assert kc2 == kc
                nc.tensor.ldweights(ident[:])  # HAM keep-warm, no output
                rhs = stagep.tile([128, SHARD], BF16, tag="rhs")
                nc.vector.tensor_tensor(
                    out=rhs[0:kc, :], in0=st_ps[0:kc, :], in1=rdr[0:kc, :],
                    op=OP.add,
                )
                pend.append((ci, rhs, kc))
                if len(pend) > PEND_DEPTH:
                    emit_mm(*pend.pop(0))

            for ci0 in range(min(RD_LOOKAHEAD, N_CHUNKS)):
                emit_rd(ci0)

            ci_iter = 0
            for t in range(N_TAIL):
                nf = min(TAIL_GROUP, F - TAIL_GROUP * t)
                # all the group's gathers first (writers must be emitted
                # before any reader for Tile dep tracking)
                for u in range(nf):
                    j = TAIL_GROUP * t + u
                    for s in range(NS):
                        nc.gpsimd.indirect_dma_start(
                            out=g[s][:, j * ROW:(j + 1) * ROW],
                            out_offset=None,
                            in_=g2_d[:, :],
                            in_offset=bass.IndirectOffsetOnAxis(
                                ap=idx[:, s * F + j:s * F + j + 1], axis=0
                            ),
                        )
                if part0 is None:
                    part0 = list(g[0][:].ap[0])
                # fm2 cross terms for field j against fields i<j:
                #   Spart[:, j] = sum_{i<j,d} g[p, j, 16i+d] * g[p, i, 16j+d]
                for u in range(nf):
                    j = TAIL_GROUP * t + u
                    if j == 0:
                        continue
                    for s in range(NS):
                        g_s = g[s]
                        in0 = bass.AP(
                            g_s[:].tensor,
                            g_s[:].offset + j * ROW + FEAT_OFF,
                            [part0, [D, j], [1, D]],
                        )
                        in1 = bass.AP(
                            g_s[:].tensor,
                            g_s[:].offset + FEAT_OFF + D * j,
                            [part0, [ROW, j], [1, D]],
                        )
                        nc.vector.scalar_tensor_tensor(
                            out=scr[:, 0:j * D].rearrange(
                                "p (i d) -> p i d", d=D),
                            in0=in0, scalar=1.0, in1=in1,
                            op0=OP.mult, op1=OP.mult,
                            accum_out=Spart[:, s * F + j:s * F + j + 1],
                        )
                for _ in range(3 * nf + 1):
                    emit_chunk(ci_iter)
                    ci_iter += 1
            assert ci_iter == N_CHUNKS
            while pend:
                emit_mm(*pend.pop(0))

            # ---- fm1 (embedding part) and fm2 reduction ----
            fm1e = small.tile([128, NS], F32, tag="fm1e")
            S_acc = small.tile([128, NS], F32, tag="fm2S")
            for s in range(NS):
                fm1_ap = g[s][:].rearrange("p (j e) -> p j e", j=F)[:, :, 0]
                nc.vector.tensor_reduce(
                    out=fm1e[:, s:s + 1], in_=fm1_ap,
                    axis=mybir.AxisListType.X, op=OP.add,
                )
                nc.vector.tensor_reduce(
                    out=S_acc[:, s:s + 1],
                    in_=Spart[:, s * F + 1:(s + 1) * F],
                    axis=mybir.AxisListType.X, op=OP.add,
                )

            # ---- BN1 stats + allreduce ----
            stats1 = small.tile([128, 4], F32, tag="stats1")
            sq_scr = scrp.tile([128, SHARD], F32, tag="sq")
            for h in range(2):
                nc.vector.tensor_reduce(
                    out=stats1[:, h:h + 1], in_=h1_ps[h][:],
                    axis=mybir.AxisListType.X, op=OP.add,
                )
                nc.scalar.activation(
                    out=sq_scr[:], in_=h1_ps[h][:], func=AF.Square,
                    accum_out=stats1[:, 2 + h:3 + h],
                )
            b1_in = dram.tile([128, 4], F32, tag="b1i")
            b1_out = dram.tile([128, 4], F32, tag="b1o")
            nc.sync.dma_start(b1_in[:], stats1[:])
            nc.gpsimd.collective_compute(
                "AllReduce", OP.add, replica_groups=groups,
                ins=[b1_in.opt()], outs=[b1_out.opt()],
            )
            stats1g = small.tile([128, 4], F32, tag="stats1g")
            nc.sync.dma_start(stats1g[:], b1_out[:])

            def bn_scale_bias(statsg, col_s, col_q, gamma, beta, ncols):
                # returns (scale, bias) [128, ncols] f32
                mean = small.tile([128, ncols], F32, tag="bn_mean")
                var = small.tile([128, ncols], F32, tag="bn_var")
                scale = small.tile([128, ncols], F32, tag="bn_scale")
                bias = small.tile([128, ncols], F32, tag="bn_bias")
                tmp = small.tile([128, ncols], F32, tag="bn_tmp")
                nc.vector.tensor_scalar_mul(
                    mean[:], statsg[:, col_s:col_s + ncols], 1.0 / BS)
                nc.vector.tensor_scalar_mul(
                    var[:], statsg[:, col_q:col_q + ncols], 1.0 / BS)
                nc.vector.tensor_tensor(
                    out=tmp[:], in0=mean[:], in1=mean[:], op=OP.mult)
                nc.vector.tensor_tensor(
                    out=var[:], in0=var[:], in1=tmp[:], op=OP.subtract)
                nc.vector.tensor_scalar_add(var[:], var[:], EPS)
                nc.vector.reciprocal(tmp[:], var[:])
                nc.scalar.activation(out=tmp[:], in_=tmp[:], func=AF.Sqrt)
                nc.vector.tensor_tensor(
                    out=scale[:], in0=gamma[:], in1=tmp[:], op=OP.mult)
                nc.vector.tensor_tensor(
                    out=tmp[:], in0=mean[:], in1=scale[:], op=OP.mult)
                nc.vector.tensor_tensor(
                    out=bias[:], in0=beta[:], in1=tmp[:], op=OP.subtract)
                return scale, bias

            sc1, bi1 = bn_scale_bias(stats1g, 0, 2, bn1g, bn1b, 2)
            h1r = persist.tile([128, 2, SHARD], BF16, tag="h1r")
            for h in range(2):
                nc.scalar.activation(
                    out=h1r[:, h, :], in_=h1_ps[h][:], func=AF.Relu,
                    bias=bi1[:, h:h + 1], scale=sc1[:, h:h + 1],
                )

            # ---- layer 2 ----
            h2_ps = ps_small.tile([128, SHARD], F32, tag="h2")
            for h in range(2):
                nc.tensor.matmul(
                    h2_ps[:],
                    lhsT=w2[:, h * 128:(h + 1) * 128],
                    rhs=h1r[:, h, :],
                    start=(h == 0), stop=(h == 1),
                )
            stats2 = small.tile([128, 2], F32, tag="stats2")
            nc.vector.tensor_reduce(
                out=stats2[:, 0:1], in_=h2_ps[:],
                axis=mybir.AxisListType.X, op=OP.add,
            )
            sq_scr2 = scrp.tile([128, SHARD], F32, tag="sq")
            nc.scalar.activation(
                out=sq_scr2[:], in_=h2_ps[:], func=AF.Square,
                accum_out=stats2[:, 1:2],
            )
            b2_in = dram.tile([128, 2], F32, tag="b2i")
            b2_out = dram.tile([128, 2], F32, tag="b2o")
            nc.sync.dma_start(b2_in[:], stats2[:])
            nc.gpsimd.collective_compute(
                "AllReduce", OP.add, replica_groups=groups,
                ins=[b2_in.opt()], outs=[b2_out.opt()],
            )
            stats2g = small.tile([128, 2], F32, tag="stats2g")
            nc.sync.dma_start(stats2g[:], b2_out[:])
            sc2, bi2 = bn_scale_bias(stats2g, 0, 1, bn2g, bn2b, 1)
            h2r = persist.tile([128, SHARD], BF16, tag="h2r")
            nc.scalar.activation(
                out=h2r[:], in_=h2_ps[:], func=AF.Relu,
                bias=bi2[:, 0:1], scale=sc2[:, 0:1],
            )

            # ---- heads ----
            for s in range(NS):
                nc.tensor.matmul(
                    head_ps[:, s:s + 1],
                    lhsT=h2r[:, s * 128:(s + 1) * 128],
                    rhs=wout[:],
                    start=True, stop=True,
                )

            tot = small.tile([128, NS], F32, tag="tot")
            res = small.tile([128, NS], F32, tag="res")
            nc.vector.tensor_tensor(
                out=tot[:], in0=fm1e[:], in1=head_ps[:, 0:NS], op=OP.add)
            nc.vector.tensor_tensor(
                out=tot[:], in0=tot[:], in1=head_ps[:, NS:2 * NS], op=OP.add)
            nc.vector.tensor_tensor(
                out=tot[:], in0=tot[:], in1=S_acc[:], op=OP.add)
            nc.scalar.activation(
                out=res[:], in_=tot[:], func=AF.Sigmoid,
                bias=c0[:, 0:1], scale=1.0,
            )
            out_ap = out_d[:, :].rearrange("(s p) o -> p (s o)", p=128)
            nc.sync.dma_start(out_ap, res[:])

            if debug_taps:
                dbg = scrp.tile([128, NS * F + 40], F32, tag="dbg")
                nc.vector.tensor_copy(dbg[:, 0:NS * F], Spart[:])
                nc.vector.tensor_copy(
                    dbg[:, NS * F:NS * F + NS], fm1e[:])
                nc.vector.tensor_copy(
                    dbg[:, NS * F + NS:NS * F + 2 * NS], S_acc[:])
                nc.vector.tensor_copy(
                    dbg[:, NS * F + 8:NS * F + 12], stats1[:])
                nc.vector.tensor_copy(
                    dbg[:, NS * F + 12:NS * F + 16], stats1g[:])
                nc.vector.tensor_copy(
                    dbg[:, NS * F + 16:NS * F + 16 + 2 * NS],
                    head_ps[:, 0:2 * NS])
                o = NS * F + 24
                nc.vector.tensor_copy(dbg[:, o:o + 2], sc1[:])
                nc.vector.tensor_copy(dbg[:, o + 2:o + 4], bi1[:])
                nc.vector.tensor_copy(dbg[:, o + 4:o + 6], stats2[:])
                nc.vector.tensor_copy(dbg[:, o + 6:o + 7], sc2[:])
                nc.vector.tensor_copy(dbg[:, o + 7:o + 8], bi2[:])
                nc.sync.dma_start(dbg_d[:, :], dbg[:])
                dbg2 = scrp.tile([128, 12], F32, tag="dbg2")
                nc.vector.tensor_copy(dbg2[:, 0:2], h1r[:, 0, 0:2])
                nc.vector.tensor_copy(dbg2[:, 2:4], h1r[:, 1, 0:2])
                nc.vector.tensor_copy(dbg2[:, 4:6], h2_ps[:, 0:2])
                nc.vector.tensor_copy(dbg2[:, 6:8], h2r[:, 0:2])
                nc.sync.dma_start(dbg2_d[:, :], dbg2[:])
                dbg_h = scrp.tile([128, 2 * SHARD], F32, tag="dbgh")
                for h in range(2):
                    nc.vector.tensor_copy(
                        dbg_h[:, h * SHARD:(h + 1) * SHARD], h1_ps[h][:])
                nc.sync.dma_start(dbg_h_d[:, :], dbg_h[:])

    if split_waits:
        _split_multiwaits(nc)
    return nc


_NC_CACHE = None


def _get_nc():
    global _NC_CACHE
    if _NC_CACHE is None:
        _NC_CACHE = build_program()
    return _NC_CACHE


def make_in_maps(X_sparse, X_dense, fm1_emb, bias, fm1_dense_W, fm1_dense_b,
                 emb_tables, dense_W, dense_b,
                 W1, b1, g1, beta1, W2, b2, g2, beta2, Wout, bout):
    bf16 = ml_dtypes.bfloat16
    f32 = np.float32

    g2t = np.zeros((V, ROW), dtype=bf16)
    g2t[:, 0] = fm1_emb[:, 0].astype(bf16)
    g2t[:, FEAT_OFF:FEAT_OFF + FD] = (
        np.ascontiguousarray(emb_tables.transpose(1, 0, 2)).reshape(V, FD)
        .astype(bf16)
    )

    # W1 permuted to g-order (field-major) rows, chunk-packed.
    W1p = np.ascontiguousarray(
        W1.reshape(H1, F, F, D).transpose(2, 1, 3, 0)
    ).reshape(DNN_IN, H1)
    # dense_W and dense_b in the same g-order.
    dWr = np.ascontiguousarray(
        dense_W.reshape(F, F, D, NDENSE).transpose(1, 0, 2, 3)
    ).reshape(DNN_IN, NDENSE)
    dbr = np.ascontiguousarray(
        dense_b.reshape(F, F, D).transpose(1, 0, 2)
    ).reshape(DNN_IN)

    w1k = np.zeros((N_CHUNKS, 128, H1), dtype=f32)
    dwrk = np.zeros((NDENSE + 1, N_CHUNKS * 128), dtype=bf16)
    for ci in range(N_CHUNKS):
        rows = _chunk_rows(ci)
        w1k[ci, 0:len(rows)] = W1p[rows]
        dwrk[0:NDENSE, ci * 128:ci * 128 + len(rows)] = dWr[rows].T.astype(bf16)
        dwrk[NDENSE, ci * 128:ci * 128 + len(rows)] = dbr[rows].astype(bf16)
    w1h = np.ascontiguousarray(w1k.transpose(1, 0, 2)).reshape(
        128, N_CHUNKS * H1).astype(bf16)

    w2h = np.ascontiguousarray(
        W2.T.reshape(2, 128, H2).transpose(1, 0, 2)
    ).reshape(128, H1).astype(bf16)
    wouth = Wout.reshape(H2, 1).astype(bf16) if Wout.shape == (H2, 1) else \
        Wout.T.astype(bf16)
    fm1wh = fm1_dense_W.T.astype(bf16)  # [13, 1]

    bn1gh = np.ascontiguousarray(g1.reshape(2, 128).T).astype(f32)
    bn1bh = np.ascontiguousarray(beta1.reshape(2, 128).T).astype(f32)
    bn2gh = g2.reshape(128, 1).astype(f32)
    bn2bh = beta2.reshape(128, 1).astype(f32)
    c0h = np.full((128, 1),
                  float(bias[0]) + float(fm1_dense_b[0]) + float(bout[0]),
                  dtype=f32)

    Xg = (X_sparse.astype(np.int64) +
          (np.arange(F, dtype=np.int64) * V_FIELD)[None, :]).astype(np.int32)

    in_maps = []
    for c in range(N_CORES):
        sl = slice(c * SHARD, (c + 1) * SHARD)
        xg_c = Xg[sl]                       # [512, 26]
        idx_c = np.zeros((128, NS * F), dtype=np.int32)
        for s in range(NS):
            idx_c[:, s * F:(s + 1) * F] = xg_c[s * 128:(s + 1) * 128, :]
        xdt_c = np.ones((NDENSE + 1, SHARD), dtype=bf16)
        xdt_c[0:NDENSE] = X_dense[sl].T.astype(bf16)
        in_maps.append({
            "g2": g2t, "idx": idx_c, "w1": w1h, "dwrk": dwrk, "xdt": xdt_c,
            "w2": w2h, "wout": wouth, "fm1w": fm1wh,
            "bn1g": bn1gh, "bn1b": bn1bh, "bn2g": bn2gh, "bn2b": bn2bh,
            "c0": c0h,
        })
    return in_maps


def kernel(**inputs):
    nc = _get_nc()
    in_maps = make_in_maps(**{k: np.asarray(v) for k, v in inputs.items()})
    res = run_bass_kernel_spmd(
        nc, in_maps, core_ids=list(range(N_CORES)),
        trace=bool(int(os.environ.get("DFM_TRACE", "0"))),
    )
    out = np.concatenate([res.results[c]["out"] for c in range(N_CORES)], axis=0)
    kernel.last_results = res
    return out.astype(np.float32)
